# revision 26
# baseline (speedup 1.0000x reference)
"""Trainium2 Bass kernel for nn_BlinkSplitCNN (dense_cnn, memory-bound).

Model: per-timestep Conv1D (center tap) -> tanh -> two MLP heads (eye + blink)
with inference-mode BatchNorm folded into the adjacent dense layers on host.

Strategy (pure data parallel, 8 cores x 2048 batch rows):
  - x is pre-transposed AND pre-cast to bf16 on the host into feature-major
    slabs laid out [128 part, total_cols]: the device reads 31.5 MB/core
    (vs 63 MB f32) with plain HWDGE DMAs at full per-partition contiguity.
    The kernel is paced end-to-end by this stream (~377 GB/s measured).
  - Variable slab plan: tiny slabs open group 0 (first conv matmul ~1us),
    15-chunk slabs in steady state, tiny slabs close group 3 so the
    dense tail starts as early as possible.
  - The conv einsum 'bwf,wfk->bwk' is a block-diagonal [7680 -> 192] matmul:
    60 K-chunks of [128, 128] bf16 weights (FWL) accumulate into 2 PSUM
    banks per group (rows 0..95 = 96 conv outputs per half, batch 512 free).
  - Dense layers all bf16, K-chunked stationary weights, N=512; staged
    across the next group's conv slabs so the in-order PE never waits on
    the scalar engine. Group 3 additionally split-K: the kc=0 half of
    e1/b1 runs as soon as conv half 0 closes, shortening the tail chain.
  - ~40 junk matmuls at t=0 flip the PE HAM clock gate to 2.4 GHz during
    the DMA fill.
  - Output written feature-major [121, 2048] bf16 per core; host upcasts
    and transposes.
"""

import os
import numpy as np
import ml_dtypes

B, W, F = 16384, 64, 120
WF = W * F            # 7680
W3 = W * 3            # 192
NCORES = 8
BL = B // NCORES      # 2048 rows per core
GROUP = 512           # batch rows per pipeline group (one PSUM bank of f32)
NGROUP = BL // GROUP  # 4
NCHUNK = WF // 128    # 60 K-chunks of 128 (w,f) rows
CPB = NCHUNK // 2     # 30 chunks accumulate per conv PSUM bank
EPS = 1e-3

# chunks per DMA slab, per group (sums to 60 each). Big slabs maximize DMA
# rate (the stream paces the kernel); tiny slabs close group 3 so the tail
# dense chain starts ~0.4us after the last x byte.
SLAB_PLAN = [
    [5, 10, 15, 15, 15],
    [15, 15, 15, 15],
    [15, 15, 15, 15],
    [15, 15, 15, 12, 2, 1],
]
CW0 = 10  # conv-weight chunks shipped on the sync ring ahead of slab 0
XCOLS = NGROUP * NCHUNK * GROUP  # total x columns per partition row

_PROGRAM = None
LAST_EXEC_NS = None
LAST_RESULTS = None


def _build_program():
    import concourse.mybir as mybir
    import concourse.tile as tile
    import concourse.bass as bass
    from concourse import bacc

    dt = mybir.dt
    AF = mybir.ActivationFunctionType

    nc = bacc.Bacc(None, target_bir_lowering=False)

    x_d = nc.dram_tensor("x", [128, XCOLS], dt.bfloat16, kind="ExternalInput")
    # conv lhsT chunks, 96 real columns each (no pad: FWL is off under
    # --enable-ldw-opt=false, so padding to 128 would only add HBM bytes)
    cw_d = nc.dram_tensor("cw", [128, NCHUNK * 96], dt.bfloat16, kind="ExternalInput")
    we1_d = nc.dram_tensor("we1", [96, 2, 192], dt.bfloat16, kind="ExternalInput")
    we2_d = nc.dram_tensor("we2", [96, 2, 240], dt.bfloat16, kind="ExternalInput")
    we3_d = nc.dram_tensor("we3", [120, 2, 120], dt.bfloat16, kind="ExternalInput")
    wb1_d = nc.dram_tensor("wb1", [96, 2, 64], dt.bfloat16, kind="ExternalInput")
    wb2_d = nc.dram_tensor("wb2", [64, 32], dt.bfloat16, kind="ExternalInput")
    wb3_d = nc.dram_tensor("wb3", [32, 1], dt.bfloat16, kind="ExternalInput")
    bias_d = nc.dram_tensor("bias", [120, 10], dt.float32, kind="ExternalInput")
    y_d = nc.dram_tensor("y", [121, BL], dt.bfloat16, kind="ExternalOutput")

    with tile.TileContext(nc) as tc:
        with (
            tc.tile_pool(name="const", bufs=1) as const,
            tc.tile_pool(name="xs", bufs=6) as xpool,
            tc.tile_pool(name="xtail", bufs=1) as xtail,
            tc.tile_pool(name="acts", bufs=4) as actp,
            tc.tile_pool(name="outp", bufs=2) as outp,
            tc.tile_pool(name="psC", bufs=4, space=bass.MemorySpace.PSUM) as psC,
            tc.tile_pool(name="psD", bufs=4, space=bass.MemorySpace.PSUM) as psD,
        ):
            # The first CW0 conv-weight chunks ride the sync ring AHEAD of
            # slab 0 (the scalar ring only gets ~1/3 of the SDMA round-robin
            # while x streams, which starved the first conv matmul of its
            # weights for ~15us); everything else on the scalar ring.
            cw = const.tile([128, NCHUNK * 96], dt.bfloat16)
            nc.sync.dma_start(out=cw[:, 0:CW0 * 96], in_=cw_d[:, 0:CW0 * 96])
            bias = const.tile([120, 10], dt.float32)
            nc.scalar.dma_start(out=bias, in_=bias_d[:])
            nc.scalar.dma_start(out=cw[:, CW0 * 96:CPB * 96],
                                in_=cw_d[:, CW0 * 96:CPB * 96])
            nc.scalar.dma_start(out=cw[:, CPB * 96:], in_=cw_d[:, CPB * 96:])
            we1 = const.tile([96, 2, 192], dt.bfloat16)
            nc.scalar.dma_start(out=we1, in_=we1_d[:])
            we2 = const.tile([96, 2, 240], dt.bfloat16)
            nc.scalar.dma_start(out=we2, in_=we2_d[:])
            we3 = const.tile([120, 2, 120], dt.bfloat16)
            nc.scalar.dma_start(out=we3, in_=we3_d[:])
            wb1 = const.tile([96, 2, 64], dt.bfloat16)
            nc.scalar.dma_start(out=wb1, in_=wb1_d[:])
            wb2 = const.tile([64, 32], dt.bfloat16)
            nc.scalar.dma_start(out=wb2, in_=wb2_d[:])
            wb3 = const.tile([32, 1], dt.bfloat16)
            nc.scalar.dma_start(out=wb3, in_=wb3_d[:])

            # ~40 back-to-back junk matmuls (~3.5us at the cold 1.2 GHz clock)
            # flip the PE HAM clock gate to 8/8 while the first x slab is
            # still in flight, so real conv work starts at 2.4 GHz
            warm = const.tile([128, 128], dt.bfloat16)
            nc.vector.memset(warm, 0.0)
            pW = psD.tile([128, 128], dt.float32, tag="psD", name="pWarm")
            for _ in range(40):
                nc.tensor.matmul(pW, warm, warm, start=True, stop=True)

            def make_dense(g, comb, e1_partial=None):
                # Dense head for group g in 3 stages; stage s is emitted
                # after conv slab s of group g+1 so every matmul's
                # ACT-produced input is a full slab (~4us of PE work) old
                # by the time the in-order PE reaches it. For the last
                # group, e1_partial holds pre-accumulated kc=0 psD tiles
                # and the stages run back-to-back in the tail.
                st = {}

                def stage0():
                    # e1 (tanh) + b1 (tanh) -- inputs: comb
                    st["e1s"] = []
                    parts = e1_partial or {}
                    for m in range(2):
                        if ("e1", m) in parts:
                            p = parts[("e1", m)]
                            nc.tensor.matmul(p, we1[:, 1, m * 96:(m + 1) * 96],
                                             comb[1], start=False, stop=True)
                        else:
                            p = psD.tile([96, GROUP], dt.float32, tag="psD")
                            for kc in range(2):
                                nc.tensor.matmul(p, we1[:, kc, m * 96:(m + 1) * 96],
                                                 comb[kc], start=(kc == 0),
                                                 stop=(kc == 1))
                        t = actp.tile([96, GROUP], dt.bfloat16, tag="e1s")
                        nc.scalar.activation(t, p, AF.Tanh, bias=bias[0:96, 2 + m:3 + m])
                        st["e1s"].append(t)
                    if ("b1", 0) in parts:
                        p = parts[("b1", 0)]
                        nc.tensor.matmul(p, wb1[:, 1, :], comb[1], start=False, stop=True)
                    else:
                        p = psD.tile([64, GROUP], dt.float32, tag="psD")
                        for kc in range(2):
                            nc.tensor.matmul(p, wb1[:, kc, :], comb[kc],
                                             start=(kc == 0), stop=(kc == 1))
                    b1s = actp.tile([64, GROUP], dt.bfloat16, tag="b1s")
                    nc.scalar.activation(b1s, p, AF.Tanh, bias=bias[0:64, 7:8])
                    st["b1s"] = b1s

                def stage1():
                    # e2 + b2 (tanh) -- inputs: e1s, b1s. e2's bias is folded
                    # into e3's on host, so its evacuation is a pure copy on
                    # the otherwise-idle vector engine (keeps the tail's
                    # critical path off the serial ACT queue)
                    st["e2s"] = []
                    for m in range(2):
                        p = psD.tile([120, GROUP], dt.float32, tag="psD")
                        for kc in range(2):
                            nc.tensor.matmul(p, we2[:, kc, m * 120:(m + 1) * 120],
                                             st["e1s"][kc], start=(kc == 0), stop=(kc == 1))
                        t = actp.tile([120, GROUP], dt.bfloat16, tag="e2s")
                        nc.vector.tensor_copy(out=t, in_=p)
                        st["e2s"].append(t)
                    p = psD.tile([32, GROUP], dt.float32, tag="psD")
                    nc.tensor.matmul(p, wb2[:, :], st["b1s"], start=True, stop=True)
                    b2s = actp.tile([32, GROUP], dt.bfloat16, tag="b2s")
                    nc.scalar.activation(b2s, p, AF.Tanh, bias=bias[0:32, 8:9])
                    st["b2s"] = b2s

                def stage2():
                    # e3 + store, then b3 (sigmoid) + store -- the eye head
                    # is the long dependency chain, so its store is issued
                    # before the blink matmuls
                    outt = outp.tile([120, GROUP], dt.bfloat16, tag="out")
                    p = psD.tile([120, GROUP], dt.float32, tag="psD")
                    for kc in range(2):
                        nc.tensor.matmul(p, we3[:, kc, :], st["e2s"][kc],
                                         start=(kc == 0), stop=(kc == 1))
                    nc.scalar.activation(outt, p, AF.Identity, bias=bias[0:120, 6:7])
                    nc.scalar.dma_start(out=y_d[0:120, g * GROUP:(g + 1) * GROUP],
                                        in_=outt)
                    p = psD.tile([1, GROUP], dt.float32, tag="psD")
                    nc.tensor.matmul(p, wb3[:, :], st["b2s"], start=True, stop=True)
                    bout = outp.tile([1, GROUP], dt.bfloat16, tag="bout")
                    nc.scalar.activation(bout, p, AF.Sigmoid, bias=bias[0:1, 9:10])
                    nc.scalar.dma_start(out=y_d[120:121, g * GROUP:(g + 1) * GROUP],
                                        in_=bout)

                return [stage0, stage1, stage2]

            pending = []
            col = 0
            for g in range(NGROUP):
                plan = SLAB_PLAN[g]
                last = g == NGROUP - 1
                pC = [psC.tile([128, GROUP], dt.float32, tag="psC", name=f"pC{g}_{h}")
                      for h in range(2)]
                comb = [actp.tile([96, GROUP], dt.bfloat16, tag="comb",
                                  name=f"comb{g}_{h}")
                        for h in range(2)]
                e1_partial = {}
                c = 0
                for s, nch in enumerate(plan):
                    pool = xpool if nch == 15 else xtail
                    xs = pool.tile([128, nch * GROUP], dt.bfloat16, tag=f"x{nch}",
                                   name=f"xs{g}_{s}")
                    nc.sync.dma_start(out=xs, in_=x_d[:, col:col + nch * GROUP])
                    col += nch * GROUP
                    for cl in range(nch):
                        h, ci = divmod(c, CPB)
                        nc.tensor.matmul(
                            pC[h][0:96, :],
                            cw[:, c * 96:(c + 1) * 96],
                            xs[:, cl * GROUP:(cl + 1) * GROUP],
                            start=(ci == 0),
                            stop=(ci == CPB - 1),
                        )
                        if ci == CPB - 1:
                            # evacuate this half through tanh(+conv bias) as
                            # soon as its accumulation closes
                            nc.scalar.activation(comb[h], pC[h][0:96, :], AF.Tanh,
                                                 bias=bias[0:96, h:h + 1])
                        c += 1
                    if pending and s < len(pending):
                        pending[s]()
                    if last and s == 2:
                        # kc=0 halves of e1/b1 for the tail group; the psD
                        # tiles stay live until stage0 closes them (no psD
                        # allocations occur in between, so the bufs=4 ring
                        # cannot recycle them early)
                        for m in range(2):
                            p = psD.tile([96, GROUP], dt.float32, tag="psD",
                                         name=f"e1p{m}")
                            nc.tensor.matmul(p, we1[:, 0, m * 96:(m + 1) * 96],
                                             comb[0], start=True, stop=False)
                            e1_partial[("e1", m)] = p
                        p = psD.tile([64, GROUP], dt.float32, tag="psD", name="b1p")
                        nc.tensor.matmul(p, wb1[:, 0, :], comb[0],
                                         start=True, stop=False)
                        e1_partial[("b1", 0)] = p
                pending = make_dense(g, comb, e1_partial if last else None)
            for stage in pending:
                stage()

    nc.compile()
    return nc


def _get_program():
    global _PROGRAM
    if _PROGRAM is None:
        _PROGRAM = _build_program()
    return _PROGRAM


def _fold_bn(g, b, m, v, W_, bias):
    s = (g.astype(np.float64) / np.sqrt(v.astype(np.float64) + EPS))
    t = b.astype(np.float64) - m.astype(np.float64) * s
    Wf = W_.astype(np.float64) * s[:, None]
    bf = bias.astype(np.float64) + t @ W_.astype(np.float64)
    return Wf, bf


def _prep_weights(i):
    bf16 = ml_dtypes.bfloat16
    f32 = np.float32

    # Block-diagonal conv weight [7680, 192]; chunk c of 128 rows hits the
    # 96-column group c // 30 (chunks align with w groups since 30*128 = 32*120).
    BD = np.zeros((WF, W3), np.float64)
    conv_w = i["conv_w"].astype(np.float64)
    for w in range(W):
        BD[w * F:(w + 1) * F, w * 3:(w + 1) * 3] = conv_w[w]
    cw = np.zeros((128, NCHUNK * 96), np.float64)
    for c in range(NCHUNK):
        g = c // CPB
        cw[:, c * 96:(c + 1) * 96] = BD[c * 128:(c + 1) * 128, g * 96:(g + 1) * 96]

    W1e, b1e = _fold_bn(i["e_g1"], i["e_b1"], i["e_m1"], i["e_v1"], i["e_d1_w"], i["e_d1_b"])
    W2e, b2e = _fold_bn(i["e_g2"], i["e_b2"], i["e_m2"], i["e_v2"], i["e_d2_w"], i["e_d2_b"])
    W3e, b3e = i["e_d3_w"].astype(np.float64), i["e_d3_b"].astype(np.float64)
    # e2's bias folds into e3's (e3 is linear in e2): its evacuation becomes
    # a pure PSUM->SBUF copy the vector engine can do
    b3e = b3e + b2e @ W3e
    b2e = np.zeros_like(b2e)
    W1b, b1b = _fold_bn(i["b_g1"], i["b_b1"], i["b_m1"], i["b_v1"], i["b_d1_w"], i["b_d1_b"])
    W2b, b2b = _fold_bn(i["b_g2"], i["b_b2"], i["b_m2"], i["b_v2"], i["b_d2_w"], i["b_d2_b"])
    W3b, b3b = i["b_d3_w"].astype(np.float64), i["b_d3_b"].astype(np.float64)

    # dense lhsT layouts: [96 (K rows), 2 (K chunk), M]
    we1 = np.stack([W1e[0:96, :], W1e[96:192, :]], axis=0).transpose(1, 0, 2)
    we2 = np.stack([W2e[0:96, :], W2e[96:192, :]], axis=0).transpose(1, 0, 2)
    we3 = np.stack([W3e[0:120, :], W3e[120:240, :]], axis=0).transpose(1, 0, 2)
    wb1 = np.stack([W1b[0:96, :], W1b[96:192, :]], axis=0).transpose(1, 0, 2)

    bias = np.zeros((120, 10), np.float64)
    cb = i["conv_b"].astype(np.float64).reshape(-1)  # [(w,k)] -> 192
    bias[0:96, 0] = cb[0:96]
    bias[0:96, 1] = cb[96:192]
    bias[0:96, 2] = b1e[0:96]
    bias[0:96, 3] = b1e[96:192]
    bias[0:120, 4] = b2e[0:120]
    bias[0:120, 5] = b2e[120:240]
    bias[0:120, 6] = b3e
    bias[0:64, 7] = b1b
    bias[0:32, 8] = b2b
    bias[0:1, 9] = b3b

    return {
        "cw": np.ascontiguousarray(cw).astype(bf16),
        "we1": np.ascontiguousarray(we1).astype(bf16),
        "we2": np.ascontiguousarray(we2).astype(bf16),
        "we3": np.ascontiguousarray(we3).astype(bf16),
        "wb1": np.ascontiguousarray(wb1).astype(bf16),
        "wb2": np.ascontiguousarray(W2b).astype(bf16),
        "wb3": np.ascontiguousarray(W3b).astype(bf16),
        "bias": np.ascontiguousarray(bias).astype(f32),
    }


def _prep_x(x):
    """[B, W, F] f32 -> per-core [128, XCOLS] bf16 slab-concatenated
    feature-major layout matching SLAB_PLAN."""
    xb = np.asarray(x, dtype=np.float32).reshape(B, WF)
    xb = xb.astype(ml_dtypes.bfloat16).view(np.uint16)  # u16 view: fast shuffles
    out = []
    for core in range(NCORES):
        buf = np.empty((128, XCOLS), np.uint16)
        col = 0
        xc = xb[core * BL:(core + 1) * BL]
        for g in range(NGROUP):
            # [512, 7680] -> chunk-major [60, 128, 512]
            xg = xc[g * GROUP:(g + 1) * GROUP].reshape(GROUP, NCHUNK, 128)
            xg = np.ascontiguousarray(xg.transpose(1, 2, 0))
            c0 = 0
            for nch in SLAB_PLAN[g]:
                w = nch * GROUP
                buf[:, col:col + w] = (
                    xg[c0:c0 + nch].transpose(1, 0, 2).reshape(128, w))
                c0 += nch
                col += w
        out.append(buf.view(ml_dtypes.bfloat16))
    return out


def kernel(**inputs):
    from concourse.bass_utils import run_bass_kernel_spmd

    global LAST_EXEC_NS, LAST_RESULTS
    nc = _get_program()
    weights = _prep_weights(inputs)
    xs = _prep_x(inputs["x"])

    in_maps = []
    for c in range(NCORES):
        m = {"x": xs[c]}
        m.update(weights)
        in_maps.append(m)

    trace = bool(int(os.environ.get("BLINK_TRACE", "0")))
    res = run_bass_kernel_spmd(nc, in_maps, list(range(NCORES)), trace=trace)
    LAST_EXEC_NS = res.exec_time_ns
    LAST_RESULTS = res
    if trace and res.exec_time_ns is not None:
        print(f"HW exec time: {res.exec_time_ns} ns")

    out = np.empty((B, F + 1), np.float32)
    for c in range(NCORES):
        out[c * BL:(c + 1) * BL, :] = res.results[c]["y"].astype(np.float32).T
    return out
